# revision 21
# baseline (speedup 1.0000x reference)
"""Trainium2 Bass kernel for nn_MultiLatentAttention (B=8, S=4096, D=2048, H=16, hd=128, L=16).

Strategy: data-parallel over batch (one batch element per core) with the giant
k/v projections algebraically eliminated, x kept resident in SBUF as bf16 so
the residual pass never re-reads HBM, and a 2-collective tail (AllToAll of the
tiny per-head context means + ReduceScatter of the rank-1 output row).

Math (per batch element):
  raw-x formulation: with alpha[t] = rsqrt(var[t]+eps), sig = 1/alpha,
    scoresT[t,hl]/sqrt(hd) = alpha[t] * (x[t,:].qhat_s[:,hl] - c_s[hl]*mu[t])
  where qhat_s = (Wk_head @ q) * g / sqrt(hd) folded host-side, c_s = sum_d.
  etilde = alpha * e = Exp(scale=alpha * psum + ln(alpha))   (one ACT op)
  u[hl,d] = sum_t etilde*x ; r = etilde^T@mu ; Z = etilde^T@sig  (=sum e)
  M = (u - r 1^T)/Z ; mbar = per-head mean over latents  [H, D]
  AllToAll routes heads {2c,2c+1} of every batch to core c; core applies its
  256-col slice of Wv*g and 256-row slice of W3 = Wlv@Wout (host-folded) for
  all 8 batches; ReduceScatter sums partials and lands row b on core b.
  y = x(bf16) + out  broadcast.  All biases folded host-side into one row.
"""

import sys
import functools
import numpy as np
import ml_dtypes

sys.path.insert(0, "/opt/trn_rl_repo")

import concourse.bass as bass
import concourse.mybir as mybir
import concourse.tile as tile
from concourse import bacc
from concourse.bass_utils import run_bass_kernel_spmd

BF = mybir.dt.bfloat16
F32 = mybir.dt.float32
AF = mybir.ActivationFunctionType

P = 128
D = 2048
KT = D // P          # 16 d-tiles
H = 16
HD = 128
L = 16
HL = H * L           # 256 score rows (h-major: hl = h*16 + l)
EPS = 1e-5
INV_SQRT_HD = 1.0 / float(np.sqrt(HD))


def _build(n_cores: int, S: int):
    NB = n_cores
    HPC = H // NB            # heads per core (2)
    SL = D // NB             # d_out slice width per core (256)
    NT = S // P              # token tiles (32)
    NQ = 4                   # u-sweep quarters
    TPQ = NT // NQ           # token tiles per quarter (8)
    NCH = D // 512           # 512-wide psum chunks (4)
    assert NT % NQ == 0 and H % NB == 0 and SL == HPC * P

    nc = bacc.Bacc(None, target_bir_lowering=False, num_devices=NB)
    groups = [list(range(NB))]

    with tile.TileContext(nc) as tc:
        with tc.tile_pool(name="dram", bufs=1, space="DRAM") as dram:
            def din(name, shape, dt):
                return dram.tile(shape, dt, kind="ExternalInput", name=name, uniquify=False)

            x_d = din("x", [S, D], BF)
            xT_d = din("xT", [P, S // P, KT, P], BF)   # host-pretransposed tiles
            qhatT_d = din("qhatT", [P, KT, HL], BF)
            cneg_d = din("cneg", [1, HL], BF)
            selmat_d = din("selmat", [P, 2, H], F32)
            wvg_d = din("wvg_s", [P, KT, SL], BF)
            w3_d = din("w3_r", [P, HPC, D], BF)
            biasf_d = din("biasf", [1, D], BF)
            y_d = dram.tile([S, D], F32, kind="ExternalOutput", name="y", uniquify=False)

            # collective bounce buffers
            mb_bounce = dram.tile([H, D], BF, name="mb_bounce")
            m2_bounce = dram.tile([H, D], BF, name="m2_bounce")
            po_bounce = dram.tile([NB, D], F32, name="po_bounce")
            ob_bounce = dram.tile([1, D], F32, name="ob_bounce")

            with (
                tc.tile_pool(name="consts", bufs=1) as consts,
                tc.tile_pool(name="resident", bufs=1) as res,
            ):
                # ---- small constants ----
                qhatT = consts.tile([P, KT, HL], BF)
                nc.sync.dma_start(qhatT[:], qhatT_d[:])
                cneg = consts.tile([1, HL], BF)
                nc.sync.dma_start(cneg[:], cneg_d[:])
                selmat = consts.tile([P, 2, H], F32)
                nc.sync.dma_start(selmat[:], selmat_d[:])
                wvg_s = consts.tile([P, KT, SL], BF)
                nc.sync.dma_start(wvg_s[:], wvg_d[:])
                w3_r = consts.tile([P, HPC, D], BF)
                nc.sync.dma_start(w3_r[:], w3_d[:])
                biasf = consts.tile([1, D], BF)
                nc.sync.dma_start(biasf[:], biasf_d[:])

                ident_bf = consts.tile([P, P], BF)
                from concourse.masks import make_identity
                make_identity(nc, ident_bf)
                onesrow_bf = consts.tile([1, P], BF)
                nc.any.memset(onesrow_bf[:], 1.0)
                ones8_bf = consts.tile([1, NB], BF)
                nc.any.memset(ones8_bf[:], 1.0)
                eps_col = consts.tile([P, 1], F32)
                nc.any.memset(eps_col[:], EPS)

                # ---- persistent state ----
                xbf = res.tile([P, NT, D], BF)        # resident x (bf16)
                musig = res.tile([P, NT, 2], BF)      # [mu, sig] per token
                u_acc = res.tile([P, 2, D], F32)
                zr_acc = res.tile([P, 2, 2, NQ], F32)  # [mh, (r,Z), quarter]

                # ================= PASS 1 =================
                with (
                    tc.tile_pool(name="epool", bufs=1) as ep,
                    tc.tile_pool(name="p1sb", bufs=1) as sb,
                    tc.tile_pool(name="p1ps", bufs=1, space="PSUM") as ps,
                    tc.tile_pool(name="p1pu", bufs=1, space="PSUM") as psu,
                    tc.tile_pool(name="p1pzr", bufs=1, space="PSUM") as pszr,
                ):
                    etil = ep.tile([P, NT, HL], BF)   # etilde per token
                    # stream all of x up front (resident bf16; host pre-cast).
                    # SWDGE lanes: keeps the HWDGE lanes free for xT loads.
                    for ti in range(NT):
                        nc.gpsimd.dma_start(xbf[:, ti, :],
                                            x_d[ti * P:(ti + 1) * P, :])
                    for q in range(NQ):
                        for lt in range(TPQ):
                            ti = q * TPQ + lt
                            # stats on bf16 x
                            bns = sb.tile([P, 4, 6], F32, tag="bns", bufs=3)
                            for a in range(4):
                                nc.vector.bn_stats(bns[:, a, :],
                                                   xbf[:, ti, a * 512:(a + 1) * 512])
                            mv = sb.tile([P, 2], F32, tag="mv", bufs=3)
                            nc.vector.bn_aggr(mv[:], bns[:])
                            sig = sb.tile([P, 1], F32, tag="sig", bufs=3)
                            nc.scalar.activation(sig[:], mv[:, 1:2], AF.Sqrt,
                                                 bias=eps_col[:])
                            alpha = sb.tile([P, 1], F32, tag="alpha", bufs=3)
                            nc.vector.reciprocal(alpha[:], sig[:])
                            nc.vector.tensor_copy(out=musig[:, ti, 0:1], in_=mv[:, 0:1])
                            nc.vector.tensor_copy(out=musig[:, ti, 1:2], in_=sig[:])
                            # transposed x tile [d, tok] (host-pretransposed)
                            xbfT = sb.tile([P, KT, P], BF, tag="xbfT", bufs=3)
                            nc.sync.dma_start(xbfT[:], xT_d[:, ti, :, :])
                            # mu as a row (PE transpose)
                            mur_ps = ps.tile([1, P], F32, tag="sc", bufs=3,
                                             name=f"mur{ti}")
                            nc.tensor.matmul(mur_ps[:], musig[:, ti, 0:1],
                                             ident_bf[:], start=True, stop=True)
                            murow = sb.tile([1, P], BF, tag="murow", bufs=3)
                            nc.vector.tensor_copy(out=murow[:], in_=mur_ps[:])
                            # scoresT accumulation: rank-1 (-c*mu) then x.qhat
                            sc_ps = ps.tile([P, HL], F32, tag="sc", bufs=3,
                                            name=f"sc{ti}")
                            nc.tensor.matmul(sc_ps[:], murow[:], cneg[:],
                                             start=True, stop=False)
                            for kt in range(KT):
                                nc.tensor.matmul(sc_ps[:], xbfT[:, kt, :],
                                                 qhatT[:, kt, :],
                                                 start=False, stop=(kt == KT - 1))
                            # etilde = alpha * exp(alpha*s)  (bf16)
                            eraw = sb.tile([P, HL], BF, tag="eraw", bufs=2)
                            nc.scalar.activation(eraw[:], sc_ps[:], AF.Exp,
                                                 scale=alpha[:])
                            nc.vector.tensor_scalar_mul(etil[:, ti, :], eraw[:],
                                                        alpha[:])

                        # ---- u / zr sweep for this quarter ----
                        for mh in range(2):
                            psum_u = psu.tile([P, D], F32, tag="u", bufs=1,
                                              name=f"u{q}_{mh}")
                            zr_ps = pszr.tile([P, 2], F32, tag="zr", bufs=1,
                                              name=f"zr{q}_{mh}")
                            for lt in range(TPQ):
                                ti = q * TPQ + lt
                                lhs = etil[:, ti, mh * P:(mh + 1) * P]
                                for nch in range(NCH):
                                    nc.tensor.matmul(
                                        psum_u[:, nch * 512:(nch + 1) * 512],
                                        lhs, xbf[:, ti, nch * 512:(nch + 1) * 512],
                                        start=(lt == 0), stop=(lt == TPQ - 1),
                                        skip_group_check=True)
                                nc.tensor.matmul(zr_ps[:], lhs, musig[:, ti, :],
                                                 start=(lt == 0), stop=(lt == TPQ - 1),
                                                 skip_group_check=True)
                            if q == 0:
                                nc.vector.tensor_copy(out=u_acc[:, mh, :],
                                                      in_=psum_u[:])
                            else:
                                nc.vector.tensor_tensor(u_acc[:, mh, :],
                                                        u_acc[:, mh, :], psum_u[:],
                                                        mybir.AluOpType.add)
                            nc.vector.tensor_copy(out=zr_acc[:, mh, :, q],
                                                  in_=zr_ps[:])

                # ================= STAGE C =================
                with tc.tile_pool(name="c_sb", bufs=1) as csb:
                    # r, Z totals; M' = (u - r)/Z in place
                    zrt = csb.tile([P, 2, 2], F32)
                    nc.vector.tensor_reduce(zrt[:], zr_acc[:], mybir.AxisListType.X,
                                            mybir.AluOpType.add)
                    rzi = csb.tile([P, 2, 1], F32)
                    nc.vector.reciprocal(rzi[:], zrt[:, :, 1:2])
                    for mh in range(2):
                        nc.vector.tensor_scalar(u_acc[:, mh, :], u_acc[:, mh, :],
                                                zrt[:, mh, 0:1], rzi[:, mh, :],
                                                mybir.AluOpType.subtract,
                                                mybir.AluOpType.mult)
                    # mbar = per-head mean [H, D] (bf16)
                    mb_bf = csb.tile([H, D], BF)
                    with tc.tile_pool(name="c_ps_mb", bufs=1, space="PSUM") as cps0:
                        mb_ps = cps0.tile([H, D], F32)
                        for mh in range(2):
                            for nch in range(NCH):
                                nc.tensor.matmul(mb_ps[:, nch * 512:(nch + 1) * 512],
                                                 selmat[:, mh, :],
                                                 u_acc[:, mh, nch * 512:(nch + 1) * 512],
                                                 start=(mh == 0), stop=(mh == 1),
                                                 skip_group_check=True)
                        nc.scalar.copy(out=mb_bf[:], in_=mb_ps[:])
                    nc.sync.dma_start(mb_bounce[:], mb_bf[:])
                    nc.gpsimd.collective_compute(
                        "AllToAll", mybir.AluOpType.bypass, replica_groups=groups,
                        ins=[mb_bounce[:].opt()], outs=[m2_bounce[:].opt()])
                    # m2 rows = (batch b, local head h2); transpose -> [d, (b,h2)]
                    m2T = csb.tile([P, KT, H], BF)
                    nc.sync.dma_start_transpose(m2T[:], m2_bounce[:])
                    # cb[h2] = mb_h @ Wvg_slice block  -> transpose -> [j, b]
                    cbT = csb.tile([P, HPC, NB], BF)
                    with tc.tile_pool(name="c_ps_cb", bufs=1, space="PSUM") as cps1:
                        for h2 in range(HPC):
                            cb_ps = cps1.tile([NB, P], F32, tag="cbps", bufs=2)
                            for kt in range(KT):
                                lhs = m2T[:, kt, :].rearrange("p (b h) -> p h b", h=HPC)
                                nc.tensor.matmul(cb_ps[:], lhs[:, h2, :],
                                                 wvg_s[:, kt, h2 * P:(h2 + 1) * P],
                                                 start=(kt == 0), stop=(kt == KT - 1))
                            cb_sb = csb.tile([NB, P], BF, tag="cbsb", bufs=2)
                            nc.scalar.copy(out=cb_sb[:], in_=cb_ps[:])
                            ct_ps = cps1.tile([P, NB], F32, tag="ctps", bufs=2)
                            nc.tensor.matmul(ct_ps[:], cb_sb[:], ident_bf[:NB, :NB],
                                             start=True, stop=True)
                            nc.scalar.copy(out=cbT[:, h2, :], in_=ct_ps[:])
                    # partial out rows for all batches: po = cb @ W3_rows + biasf/NB
                    with tc.tile_pool(name="c_ps_po", bufs=1, space="PSUM") as cps2:
                        po_ps = cps2.tile([NB, D], F32)
                        for h2 in range(HPC):
                            for nch in range(NCH):
                                nc.tensor.matmul(po_ps[:, nch * 512:(nch + 1) * 512],
                                                 cbT[:, h2, :],
                                                 w3_r[:, h2, nch * 512:(nch + 1) * 512],
                                                 start=(h2 == 0), stop=False,
                                                 skip_group_check=True)
                        for nch in range(NCH):
                            nc.tensor.matmul(po_ps[:, nch * 512:(nch + 1) * 512],
                                             ones8_bf[:],
                                             biasf[:, nch * 512:(nch + 1) * 512],
                                             start=False, stop=(nch == NCH - 1),
                                             skip_group_check=True)
                        for half in range(2):
                            po_sb = csb.tile([NB, D // 2], F32, tag="posb", bufs=1)
                            nc.scalar.copy(out=po_sb[:],
                                           in_=po_ps[:, half * 1024:(half + 1) * 1024])
                            nc.sync.dma_start(
                                po_bounce[:, half * 1024:(half + 1) * 1024], po_sb[:])
                    nc.gpsimd.collective_compute(
                        "ReduceScatter", mybir.AluOpType.add, replica_groups=groups,
                        ins=[po_bounce[:].opt()], outs=[ob_bounce[:].opt()])
                    # broadcast own out row to 128 partitions (bf16)
                    ob_bf = csb.tile([1, D], BF)
                    nc.gpsimd.dma_start(ob_bf[:], ob_bounce[:])  # f32->bf16 cast
                    obb = res.tile([P, D], BF)
                    with tc.tile_pool(name="c_ps_bc", bufs=1, space="PSUM") as cps3:
                        bc_ps = cps3.tile([P, D], F32)
                        for nch in range(NCH):
                            nc.tensor.matmul(bc_ps[:, nch * 512:(nch + 1) * 512],
                                             onesrow_bf[:],
                                             ob_bf[:, nch * 512:(nch + 1) * 512],
                                             start=True, stop=True,
                                             skip_group_check=True)
                        nc.scalar.copy(out=obb[:], in_=bc_ps[:])

                # ================= PASS 2 (residual, no x re-read) =========
                with tc.tile_pool(name="res2", bufs=1) as r2:
                    for ti in range(NT):
                        yt = r2.tile([P, D], F32, tag="yt", bufs=3)
                        eng = nc.vector if ti % 2 == 0 else nc.gpsimd
                        eng.tensor_tensor(yt[:], xbf[:, ti, :], obb[:],
                                          mybir.AluOpType.add)
                        nc.sync.dma_start(y_d[ti * P:(ti + 1) * P, :], yt[:])

    nc.compile()
    return nc


@functools.lru_cache(maxsize=2)
def _built(n_cores: int, S: int):
    return _build(n_cores, S)


def _host_prep(inputs, n_cores: int):
    """Weight folding on host. Returns (global_map, per_core_maps)."""
    NB = n_cores
    HPC = H // NB
    SL = D // NB
    f32 = np.float32
    bf16 = ml_dtypes.bfloat16

    x_all = np.ascontiguousarray(np.asarray(inputs["hidden_states"], dtype=f32))
    g = np.asarray(inputs["ln_g"], dtype=f32)
    b_ln = np.asarray(inputs["ln_b"], dtype=f32)
    lat = np.asarray(inputs["latents"], dtype=f32)
    w_lq = np.asarray(inputs["w_lq"], dtype=f32)
    b_lq = np.asarray(inputs["b_lq"], dtype=f32)
    w_k = np.asarray(inputs["w_k"], dtype=f32)
    w_v = np.asarray(inputs["w_v"], dtype=f32)
    b_v = np.asarray(inputs["b_v"], dtype=f32)
    w_lv = np.asarray(inputs["w_lv"], dtype=f32)
    b_lv = np.asarray(inputs["b_lv"], dtype=f32)
    w_out = np.asarray(inputs["w_out"], dtype=f32)
    b_out = np.asarray(inputs["b_out"], dtype=f32)

    q_full = lat @ w_lq + b_lq                      # [L, D]
    qhatT = np.empty((D, HL), f32)
    for h in range(H):
        qh = q_full[:, HD * h:HD * (h + 1)]          # [L, 128]
        qhatT[:, L * h:L * (h + 1)] = w_k[:, HD * h:HD * (h + 1)] @ qh.T
    qhatT *= g[:, None] * INV_SQRT_HD               # fold 1/sqrt(hd)
    c_vec = qhatT.sum(axis=0)                        # [HL] (already scaled)

    def tile_rows(a):  # [D, N] -> [P, KT, N] with d = t*128 + p
        return np.ascontiguousarray(a.reshape(-1, P, a.shape[-1]).transpose(1, 0, 2))

    qhatT_t = tile_rows(qhatT).astype(bf16)
    cneg = (-c_vec)[None, :].astype(bf16)

    selmat = np.zeros((P, 2, H), f32)
    for mh in range(2):
        for p in range(P):
            selmat[p, mh, (mh * P + p) // L] = 1.0 / L

    wvg = w_v * g[:, None]
    w3 = w_lv @ w_out                                # folded Wlv@Wout [D, D]
    bv_fold = b_v + b_ln @ w_v
    biasf_full = ((bv_fold @ w_lv + b_lv) @ w_out + b_out) / NB

    global_map = {
        "qhatT": qhatT_t, "cneg": cneg, "selmat": selmat,
        "biasf": np.ascontiguousarray(biasf_full[None, :].astype(bf16)),
    }
    per_core = []
    for c in range(NB):
        sl = slice(SL * c, SL * (c + 1))
        wvg_s = tile_rows(wvg[:, sl]).astype(bf16)               # [P, KT, SL]
        w3_rows = np.ascontiguousarray(
            w3[sl, :].reshape(HPC, P, D).transpose(1, 0, 2)).astype(bf16)
        xc = x_all[c].astype(bf16)
        S = xc.shape[0]
        # xT[p, ti, kt, j] = x[ti*128+j, kt*128+p]
        xT = np.ascontiguousarray(
            xc.reshape(S // P, P, D // P, P).transpose(3, 0, 2, 1))
        per_core.append({
            "x": np.ascontiguousarray(xc),
            "xT": xT, "wvg_s": wvg_s, "w3_r": w3_rows,
        })
    return global_map, per_core


def kernel(**inputs) -> np.ndarray:
    NB = 8
    x_all = np.asarray(inputs["hidden_states"])
    B, S, D_ = x_all.shape
    assert D_ == D and B == NB
    nc = _built(NB, S)
    global_map, per_core = _host_prep(inputs, NB)
    in_maps = [{**global_map, **pc} for pc in per_core]
    res = run_bass_kernel_spmd(nc, in_maps, list(range(NB)))
    out = np.stack([res.results[i]["y"] for i in range(NB)], axis=0)
    return out.astype(np.float32)


# revision 23
# speedup vs baseline: 1.1887x; 1.1887x over previous
"""Trainium2 Bass kernel for nn_MultiLatentAttention (B=8, S=4096, D=2048, H=16, hd=128, L=16).

Strategy: data-parallel over batch (one batch element per core) with the giant
k/v projections algebraically eliminated, x kept resident in SBUF as bf16 so
the residual pass never re-reads HBM, and a 2-collective tail (AllToAll of the
tiny per-head context means + ReduceScatter of the rank-1 output row).

Math (per batch element):
  raw-x formulation: with alpha[t] = rsqrt(var[t]+eps), sig = 1/alpha,
    scoresT[t,hl]/sqrt(hd) = alpha[t] * (x[t,:].qhat_s[:,hl] - c_s[hl]*mu[t])
  where qhat_s = (Wk_head @ q) * g / sqrt(hd) folded host-side, c_s = sum_d.
  etilde = alpha * e = Exp(scale=alpha * psum + ln(alpha))   (one ACT op)
  u[hl,d] = sum_t etilde*x ; r = etilde^T@mu ; Z = etilde^T@sig  (=sum e)
  M = (u - r 1^T)/Z ; mbar = per-head mean over latents  [H, D]
  AllToAll routes heads {2c,2c+1} of every batch to core c; core applies its
  256-col slice of Wv*g and 256-row slice of W3 = Wlv@Wout (host-folded) for
  all 8 batches; ReduceScatter sums partials and lands row b on core b.
  y = x(bf16) + out  broadcast.  All biases folded host-side into one row.
"""

import sys
import functools
import numpy as np
import ml_dtypes

sys.path.insert(0, "/opt/trn_rl_repo")

import concourse.bass as bass
import concourse.mybir as mybir
import concourse.tile as tile
from concourse import bacc
from concourse.bass_utils import run_bass_kernel_spmd

BF = mybir.dt.bfloat16
F32 = mybir.dt.float32
AF = mybir.ActivationFunctionType

P = 128
D = 2048
KT = D // P          # 16 d-tiles
H = 16
HD = 128
L = 16
HL = H * L           # 256 score rows (h-major: hl = h*16 + l)
EPS = 1e-5
INV_SQRT_HD = 1.0 / float(np.sqrt(HD))


def _build(n_cores: int, S: int):
    NB = n_cores
    HPC = H // NB            # heads per core (2)
    SL = D // NB             # d_out slice width per core (256)
    NT = S // P              # token tiles (32)
    NQ = 4                   # u-sweep quarters
    TPQ = NT // NQ           # token tiles per quarter (8)
    NCH = D // 512           # 512-wide psum chunks (4)
    assert NT % NQ == 0 and H % NB == 0 and SL == HPC * P

    nc = bacc.Bacc(None, target_bir_lowering=False, num_devices=NB)
    groups = [list(range(NB))]

    with tile.TileContext(nc) as tc:
        with tc.tile_pool(name="dram", bufs=1, space="DRAM") as dram:
            def din(name, shape, dt):
                return dram.tile(shape, dt, kind="ExternalInput", name=name, uniquify=False)

            x_d = din("x", [S, D], BF)
            xT_d = din("xT", [P, S // P, KT, P], BF)   # host-pretransposed tiles
            qhatT_d = din("qhatT", [P, KT, HL], BF)
            cneg_d = din("cneg", [1, HL], BF)
            selmat_d = din("selmat", [P, 2, H], F32)
            wvg_d = din("wvg_s", [P, KT, SL], BF)
            w3_d = din("w3_r", [P, HPC, D], BF)
            biasf_d = din("biasf", [1, D], BF)
            y_d = dram.tile([S, D], F32, kind="ExternalOutput", name="y", uniquify=False)

            # collective bounce buffers
            mb_bounce = dram.tile([H, D], BF, name="mb_bounce")
            m2_bounce = dram.tile([H, D], BF, name="m2_bounce")
            po_bounce = dram.tile([NB, D], F32, name="po_bounce")
            ob_bounce = dram.tile([1, D], F32, name="ob_bounce")

            with (
                tc.tile_pool(name="consts", bufs=1) as consts,
                tc.tile_pool(name="resident", bufs=1) as res,
            ):
                # ---- small constants ----
                qhatT = consts.tile([P, KT, HL], BF)
                nc.sync.dma_start(qhatT[:], qhatT_d[:])
                cneg = consts.tile([1, HL], BF)
                nc.sync.dma_start(cneg[:], cneg_d[:])
                selmat = consts.tile([P, 2, H], F32)
                nc.sync.dma_start(selmat[:], selmat_d[:])
                wvg_s = consts.tile([P, KT, SL], BF)
                nc.sync.dma_start(wvg_s[:], wvg_d[:])
                w3_r = consts.tile([P, HPC, D], BF)
                nc.sync.dma_start(w3_r[:], w3_d[:])
                biasf = consts.tile([1, D], BF)
                nc.sync.dma_start(biasf[:], biasf_d[:])

                ident_bf = consts.tile([P, P], BF)
                from concourse.masks import make_identity
                make_identity(nc, ident_bf)
                onesrow_bf = consts.tile([1, P], BF)
                nc.any.memset(onesrow_bf[:], 1.0)
                ones8_bf = consts.tile([1, NB], BF)
                nc.any.memset(ones8_bf[:], 1.0)
                eps_col = consts.tile([P, 1], F32)
                nc.any.memset(eps_col[:], EPS)

                # ---- persistent state ----
                xbf = res.tile([P, NT, D], BF)        # resident x (bf16)
                musig = res.tile([P, NT, 2], BF)      # [mu, sig] per token
                u_acc = res.tile([P, 2, D], F32)
                zr_acc = res.tile([P, 2, 2, NQ], F32)  # [mh, (r,Z), quarter]

                # ================= PASS 1 =================
                with (
                    tc.tile_pool(name="epool", bufs=1) as ep,
                    tc.tile_pool(name="p1sb", bufs=1) as sb,
                    tc.tile_pool(name="p1ps", bufs=1, space="PSUM") as ps,
                    tc.tile_pool(name="p1pu", bufs=1, space="PSUM") as psu,
                    tc.tile_pool(name="p1pzr", bufs=1, space="PSUM") as pszr,
                ):
                    etil = ep.tile([P, NT, HL], BF)   # etilde per token
                    # stream all of x up front (resident bf16; host pre-cast).
                    # SWDGE lanes: keeps the HWDGE lanes free for xT loads.
                    for ti in range(NT):
                        nc.gpsimd.dma_start(xbf[:, ti, :],
                                            x_d[ti * P:(ti + 1) * P, :])
                    for q in range(NQ):
                        for lt in range(TPQ):
                            ti = q * TPQ + lt
                            # stats on bf16 x
                            bns = sb.tile([P, 4, 6], F32, tag="bns", bufs=3)
                            for a in range(4):
                                nc.vector.bn_stats(bns[:, a, :],
                                                   xbf[:, ti, a * 512:(a + 1) * 512])
                            mv = sb.tile([P, 2], F32, tag="mv", bufs=3)
                            nc.vector.bn_aggr(mv[:], bns[:])
                            sig = sb.tile([P, 1], F32, tag="sig", bufs=3)
                            nc.scalar.activation(sig[:], mv[:, 1:2], AF.Sqrt,
                                                 bias=eps_col[:])
                            alpha = sb.tile([P, 1], F32, tag="alpha", bufs=3)
                            nc.vector.reciprocal(alpha[:], sig[:])
                            nc.vector.tensor_copy(out=musig[:, ti, 0:1], in_=mv[:, 0:1])
                            nc.vector.tensor_copy(out=musig[:, ti, 1:2], in_=sig[:])
                            # transposed x tile [d, tok] (host-pretransposed)
                            xbfT = sb.tile([P, KT, P], BF, tag="xbfT", bufs=3)
                            nc.sync.dma_start(xbfT[:], xT_d[:, ti, :, :])
                            # mu as a row (PE transpose)
                            mur_ps = ps.tile([1, P], F32, tag="sc", bufs=3,
                                             name=f"mur{ti}")
                            nc.tensor.matmul(mur_ps[:], musig[:, ti, 0:1],
                                             ident_bf[:], start=True, stop=True)
                            murow = sb.tile([1, P], BF, tag="murow", bufs=3)
                            nc.vector.tensor_copy(out=murow[:], in_=mur_ps[:])
                            # scoresT accumulation: rank-1 (-c*mu) then x.qhat
                            sc_ps = ps.tile([P, HL], F32, tag="sc", bufs=3,
                                            name=f"sc{ti}")
                            nc.tensor.matmul(sc_ps[:], murow[:], cneg[:],
                                             start=True, stop=False)
                            for kt in range(KT):
                                nc.tensor.matmul(sc_ps[:], xbfT[:, kt, :],
                                                 qhatT[:, kt, :],
                                                 start=False, stop=(kt == KT - 1))
                            # etilde = alpha * exp(alpha*s)  (bf16)
                            eraw = sb.tile([P, HL], BF, tag="eraw", bufs=2)
                            nc.scalar.activation(eraw[:], sc_ps[:], AF.Exp,
                                                 scale=alpha[:])
                            nc.vector.tensor_scalar_mul(etil[:, ti, :], eraw[:],
                                                        alpha[:])

                        # ---- u / zr sweep for this quarter ----
                        for mh in range(2):
                            psum_u = psu.tile([P, D], F32, tag="u", bufs=1,
                                              name=f"u{q}_{mh}")
                            zr_ps = pszr.tile([P, 2], F32, tag="zr", bufs=1,
                                              name=f"zr{q}_{mh}")
                            for lt in range(TPQ):
                                ti = q * TPQ + lt
                                lhs = etil[:, ti, mh * P:(mh + 1) * P]
                                for nch in range(NCH):
                                    nc.tensor.matmul(
                                        psum_u[:, nch * 512:(nch + 1) * 512],
                                        lhs, xbf[:, ti, nch * 512:(nch + 1) * 512],
                                        start=(lt == 0), stop=(lt == TPQ - 1),
                                        skip_group_check=True)
                                nc.tensor.matmul(zr_ps[:], lhs, musig[:, ti, :],
                                                 start=(lt == 0), stop=(lt == TPQ - 1),
                                                 skip_group_check=True)
                            if q == 0:
                                nc.vector.tensor_copy(out=u_acc[:, mh, :],
                                                      in_=psum_u[:])
                            else:
                                nc.vector.tensor_tensor(u_acc[:, mh, :],
                                                        u_acc[:, mh, :], psum_u[:],
                                                        mybir.AluOpType.add)
                            nc.vector.tensor_copy(out=zr_acc[:, mh, :, q],
                                                  in_=zr_ps[:])

                # ================= STAGE C =================
                with tc.tile_pool(name="c_sb", bufs=1) as csb:
                    # r, Z totals; M' = (u - r)/Z in place
                    zrt = csb.tile([P, 2, 2], F32)
                    nc.vector.tensor_reduce(zrt[:], zr_acc[:], mybir.AxisListType.X,
                                            mybir.AluOpType.add)
                    rzi = csb.tile([P, 2, 1], F32)
                    nc.vector.reciprocal(rzi[:], zrt[:, :, 1:2])
                    for mh in range(2):
                        nc.vector.tensor_scalar(u_acc[:, mh, :], u_acc[:, mh, :],
                                                zrt[:, mh, 0:1], rzi[:, mh, :],
                                                mybir.AluOpType.subtract,
                                                mybir.AluOpType.mult)
                    # mbar = per-head mean [H, D] (bf16)
                    mb_bf = csb.tile([H, D], BF)
                    with tc.tile_pool(name="c_ps_mb", bufs=1, space="PSUM") as cps0:
                        mb_ps = cps0.tile([H, D], F32)
                        for mh in range(2):
                            for nch in range(NCH):
                                nc.tensor.matmul(mb_ps[:, nch * 512:(nch + 1) * 512],
                                                 selmat[:, mh, :],
                                                 u_acc[:, mh, nch * 512:(nch + 1) * 512],
                                                 start=(mh == 0), stop=(mh == 1),
                                                 skip_group_check=True)
                        nc.scalar.copy(out=mb_bf[:], in_=mb_ps[:])
                    nc.sync.dma_start(mb_bounce[:], mb_bf[:])
                    nc.gpsimd.collective_compute(
                        "AllToAll", mybir.AluOpType.bypass, replica_groups=groups,
                        ins=[mb_bounce[:].opt()], outs=[m2_bounce[:].opt()])
                    # m2 rows = (batch b, local head h2); transpose -> [d, (b,h2)]
                    m2T = csb.tile([P, KT, H], BF)
                    nc.sync.dma_start_transpose(m2T[:], m2_bounce[:])
                    # cb[h2] = mb_h @ Wvg_slice block  -> transpose -> [j, b]
                    cbT = csb.tile([P, HPC, NB], BF)
                    with tc.tile_pool(name="c_ps_cb", bufs=1, space="PSUM") as cps1:
                        for h2 in range(HPC):
                            cb_ps = cps1.tile([NB, P], F32, tag="cbps", bufs=2)
                            for kt in range(KT):
                                lhs = m2T[:, kt, :].rearrange("p (b h) -> p h b", h=HPC)
                                nc.tensor.matmul(cb_ps[:], lhs[:, h2, :],
                                                 wvg_s[:, kt, h2 * P:(h2 + 1) * P],
                                                 start=(kt == 0), stop=(kt == KT - 1))
                            cb_sb = csb.tile([NB, P], BF, tag="cbsb", bufs=2)
                            nc.scalar.copy(out=cb_sb[:], in_=cb_ps[:])
                            ct_ps = cps1.tile([P, NB], F32, tag="ctps", bufs=2)
                            nc.tensor.matmul(ct_ps[:], cb_sb[:], ident_bf[:NB, :NB],
                                             start=True, stop=True)
                            nc.scalar.copy(out=cbT[:, h2, :], in_=ct_ps[:])
                    # partial out rows for all batches: po = cb @ W3_rows + biasf/NB
                    with tc.tile_pool(name="c_ps_po", bufs=1, space="PSUM") as cps2:
                        po_ps = cps2.tile([NB, D], F32)
                        for h2 in range(HPC):
                            for nch in range(NCH):
                                nc.tensor.matmul(po_ps[:, nch * 512:(nch + 1) * 512],
                                                 cbT[:, h2, :],
                                                 w3_r[:, h2, nch * 512:(nch + 1) * 512],
                                                 start=(h2 == 0), stop=False,
                                                 skip_group_check=True)
                        for nch in range(NCH):
                            nc.tensor.matmul(po_ps[:, nch * 512:(nch + 1) * 512],
                                             ones8_bf[:],
                                             biasf[:, nch * 512:(nch + 1) * 512],
                                             start=False, stop=(nch == NCH - 1),
                                             skip_group_check=True)
                        for half in range(2):
                            po_sb = csb.tile([NB, D // 2], F32, tag="posb", bufs=1)
                            nc.scalar.copy(out=po_sb[:],
                                           in_=po_ps[:, half * 1024:(half + 1) * 1024])
                            nc.sync.dma_start(
                                po_bounce[:, half * 1024:(half + 1) * 1024], po_sb[:])
                    nc.gpsimd.collective_compute(
                        "ReduceScatter", mybir.AluOpType.add, replica_groups=groups,
                        ins=[po_bounce[:].opt()], outs=[ob_bounce[:].opt()])
                    # broadcast own out row to 128 partitions (bf16)
                    ob_bf = csb.tile([1, D], BF)
                    nc.gpsimd.dma_start(ob_bf[:], ob_bounce[:])  # f32->bf16 cast
                    obb = res.tile([P, D], BF)
                    with tc.tile_pool(name="c_ps_bc", bufs=1, space="PSUM") as cps3:
                        bc_ps = cps3.tile([P, D], F32)
                        for nch in range(NCH):
                            nc.tensor.matmul(bc_ps[:, nch * 512:(nch + 1) * 512],
                                             onesrow_bf[:],
                                             ob_bf[:, nch * 512:(nch + 1) * 512],
                                             start=True, stop=True,
                                             skip_group_check=True)
                        nc.scalar.copy(out=obb[:], in_=bc_ps[:])

                # ================= PASS 2 (residual, no x re-read) =========
                with tc.tile_pool(name="res2", bufs=1) as r2:
                    for ti in range(NT):
                        yt = r2.tile([P, D], F32, tag="yt", bufs=3)
                        # vector is ~1.4x faster than gpsimd: give it 2 of 3
                        eng = nc.gpsimd if ti % 3 == 2 else nc.vector
                        eng.tensor_tensor(yt[:], xbf[:, ti, :], obb[:],
                                          mybir.AluOpType.add)
                        nc.sync.dma_start(y_d[ti * P:(ti + 1) * P, :], yt[:])

    nc.compile()
    return nc


@functools.lru_cache(maxsize=2)
def _built(n_cores: int, S: int):
    return _build(n_cores, S)


def _host_prep(inputs, n_cores: int):
    """Weight folding on host. Returns (global_map, per_core_maps)."""
    NB = n_cores
    HPC = H // NB
    SL = D // NB
    f32 = np.float32
    bf16 = ml_dtypes.bfloat16

    x_all = np.ascontiguousarray(np.asarray(inputs["hidden_states"], dtype=f32))
    g = np.asarray(inputs["ln_g"], dtype=f32)
    b_ln = np.asarray(inputs["ln_b"], dtype=f32)
    lat = np.asarray(inputs["latents"], dtype=f32)
    w_lq = np.asarray(inputs["w_lq"], dtype=f32)
    b_lq = np.asarray(inputs["b_lq"], dtype=f32)
    w_k = np.asarray(inputs["w_k"], dtype=f32)
    w_v = np.asarray(inputs["w_v"], dtype=f32)
    b_v = np.asarray(inputs["b_v"], dtype=f32)
    w_lv = np.asarray(inputs["w_lv"], dtype=f32)
    b_lv = np.asarray(inputs["b_lv"], dtype=f32)
    w_out = np.asarray(inputs["w_out"], dtype=f32)
    b_out = np.asarray(inputs["b_out"], dtype=f32)

    q_full = lat @ w_lq + b_lq                      # [L, D]
    qhatT = np.empty((D, HL), f32)
    for h in range(H):
        qh = q_full[:, HD * h:HD * (h + 1)]          # [L, 128]
        qhatT[:, L * h:L * (h + 1)] = w_k[:, HD * h:HD * (h + 1)] @ qh.T
    qhatT *= g[:, None] * INV_SQRT_HD               # fold 1/sqrt(hd)
    c_vec = qhatT.sum(axis=0)                        # [HL] (already scaled)

    def tile_rows(a):  # [D, N] -> [P, KT, N] with d = t*128 + p
        return np.ascontiguousarray(a.reshape(-1, P, a.shape[-1]).transpose(1, 0, 2))

    qhatT_t = tile_rows(qhatT).astype(bf16)
    cneg = (-c_vec)[None, :].astype(bf16)

    selmat = np.zeros((P, 2, H), f32)
    for mh in range(2):
        for p in range(P):
            selmat[p, mh, (mh * P + p) // L] = 1.0 / L

    wvg = w_v * g[:, None]
    w3 = w_lv @ w_out                                # folded Wlv@Wout [D, D]
    bv_fold = b_v + b_ln @ w_v
    biasf_full = ((bv_fold @ w_lv + b_lv) @ w_out + b_out) / NB

    global_map = {
        "qhatT": qhatT_t, "cneg": cneg, "selmat": selmat,
        "biasf": np.ascontiguousarray(biasf_full[None, :].astype(bf16)),
    }
    per_core = []
    for c in range(NB):
        sl = slice(SL * c, SL * (c + 1))
        wvg_s = tile_rows(wvg[:, sl]).astype(bf16)               # [P, KT, SL]
        w3_rows = np.ascontiguousarray(
            w3[sl, :].reshape(HPC, P, D).transpose(1, 0, 2)).astype(bf16)
        xc = x_all[c].astype(bf16)
        S = xc.shape[0]
        # xT[p, ti, kt, j] = x[ti*128+j, kt*128+p]
        xT = np.ascontiguousarray(
            xc.reshape(S // P, P, D // P, P).transpose(3, 0, 2, 1))
        per_core.append({
            "x": np.ascontiguousarray(xc),
            "xT": xT, "wvg_s": wvg_s, "w3_r": w3_rows,
        })
    return global_map, per_core


def kernel(**inputs) -> np.ndarray:
    NB = 8
    x_all = np.asarray(inputs["hidden_states"])
    B, S, D_ = x_all.shape
    assert D_ == D and B == NB
    nc = _built(NB, S)
    global_map, per_core = _host_prep(inputs, NB)
    in_maps = [{**global_map, **pc} for pc in per_core]
    res = run_bass_kernel_spmd(nc, in_maps, list(range(NB)))
    out = np.stack([res.results[i]["y"] for i in range(NB)], axis=0)
    return out.astype(np.float32)


# revision 32
# speedup vs baseline: 1.1972x; 1.0072x over previous
"""Trainium2 Bass kernel for nn_MultiLatentAttention (B=8, S=4096, D=2048, H=16, hd=128, L=16).

Strategy: data-parallel over batch (one batch element per core) with the giant
k/v projections algebraically eliminated, x kept resident in SBUF as bf16 so
the residual pass never re-reads HBM, and a 2-collective tail (AllToAll of the
tiny per-head context means + ReduceScatter of the rank-1 output row).

Math (per batch element):
  raw-x formulation: with alpha[t] = rsqrt(var[t]+eps), sig = 1/alpha,
    scoresT[t,hl]/sqrt(hd) = alpha[t] * (x[t,:].qhat_s[:,hl] - c_s[hl]*mu[t])
  where qhat_s = (Wk_head @ q) * g / sqrt(hd) folded host-side, c_s = sum_d.
  etilde = alpha * e = Exp(scale=alpha * psum + ln(alpha))   (one ACT op)
  u[hl,d] = sum_t etilde*x ; r = etilde^T@mu ; Z = etilde^T@sig  (=sum e)
  M = (u - r 1^T)/Z ; mbar = per-head mean over latents  [H, D]
  AllToAll routes heads {2c,2c+1} of every batch to core c; core applies its
  256-col slice of Wv*g and 256-row slice of W3 = Wlv@Wout (host-folded) for
  all 8 batches; ReduceScatter sums partials and lands row b on core b.
  y = x(bf16) + out  broadcast.  All biases folded host-side into one row.
"""

import sys
import functools
import numpy as np
import ml_dtypes

sys.path.insert(0, "/opt/trn_rl_repo")

import concourse.bass as bass
import concourse.mybir as mybir
import concourse.tile as tile
from concourse import bacc
from concourse.bass_utils import run_bass_kernel_spmd

BF = mybir.dt.bfloat16
F8 = mybir.dt.float8e4
F32 = mybir.dt.float32
AF = mybir.ActivationFunctionType
QSCALE = 128.0      # qhat is ~2e-3: scale into fp8 normal range, undo in exp

P = 128
D = 2048
KT = D // P          # 16 d-tiles
H = 16
HD = 128
L = 16
HL = H * L           # 256 score rows (h-major: hl = h*16 + l)
EPS = 1e-5
INV_SQRT_HD = 1.0 / float(np.sqrt(HD))


def _build(n_cores: int, S: int):
    NB = n_cores
    HPC = H // NB            # heads per core (2)
    SL = D // NB             # d_out slice width per core (256)
    NT = S // P              # token tiles (32)
    NQ = 4                   # u-sweep quarters
    TPQ = NT // NQ           # token tiles per quarter (8)
    NCH = D // 512           # 512-wide psum chunks (4)
    assert NT % NQ == 0 and H % NB == 0 and SL == HPC * P

    nc = bacc.Bacc(None, target_bir_lowering=False, num_devices=NB)
    groups = [list(range(NB))]

    with tile.TileContext(nc) as tc:
        with tc.tile_pool(name="dram", bufs=1, space="DRAM") as dram:
            def din(name, shape, dt):
                return dram.tile(shape, dt, kind="ExternalInput", name=name, uniquify=False)

            x_d = din("x", [S, D], BF)
            xT_d = din("xT", [P, S // P, KT, P], F8)   # host-pretransposed tiles
            qhatT_d = din("qhatT", [P, KT, HL], F8)
            cneg_d = din("cneg", [1, HL], BF)
            selmat_d = din("selmat", [P, 2, H], F32)
            wvg_d = din("wvg_s", [P, KT, SL], BF)
            w3_d = din("w3_r", [P, HPC, D], BF)
            biasf_d = din("biasf", [1, D], BF)
            y_d = dram.tile([S, D], F32, kind="ExternalOutput", name="y", uniquify=False)

            # collective bounce buffers
            mb_bounce = dram.tile([H, D], BF, name="mb_bounce")
            m2_bounce = dram.tile([H, D], BF, name="m2_bounce")
            po_bounce = dram.tile([NB, D], F32, name="po_bounce")
            ob_bounce = dram.tile([1, D], F32, name="ob_bounce")

            with (
                tc.tile_pool(name="consts", bufs=1) as consts,
                tc.tile_pool(name="resident", bufs=1) as res,
            ):
                # ---- small constants ----
                # qhat/cneg gate tile-0 scores: first on the SWDGE queue.
                # Stage-C weights are deferred until after the x loads.
                qhatT = consts.tile([P, KT, HL], F8)
                nc.gpsimd.dma_start(qhatT[:], qhatT_d[:])
                cneg = consts.tile([1, HL], BF)
                nc.gpsimd.dma_start(cneg[:], cneg_d[:])
                selmat = consts.tile([P, 2, H], F32)
                wvg_s = consts.tile([P, KT, SL], BF)
                w3_r = consts.tile([P, HPC, D], BF)
                biasf = consts.tile([1, D], BF)

                ident_bf = consts.tile([P, P], BF)
                from concourse.masks import make_identity
                make_identity(nc, ident_bf)
                onesrow_bf = consts.tile([1, P], BF)
                nc.any.memset(onesrow_bf[:], 1.0)
                ones8_bf = consts.tile([1, NB], BF)
                nc.any.memset(ones8_bf[:], 1.0)
                eps_col = consts.tile([P, 1], F32)
                nc.any.memset(eps_col[:], EPS)

                # ---- persistent state ----
                xbf = res.tile([P, NT, D], BF)        # resident x (bf16)
                musig = res.tile([P, NT, 2], BF)      # [mu, sig] per token
                u_acc = res.tile([P, 2, D], F32)
                zr_acc = res.tile([P, 2, 2, NQ], F32)  # [mh, (r,Z), quarter]

                # ================= PASS 1 =================
                with (
                    tc.tile_pool(name="epool", bufs=1) as ep,
                    tc.tile_pool(name="p1sb", bufs=1) as sb,
                    tc.tile_pool(name="p1ps", bufs=1, space="PSUM") as ps,
                    tc.tile_pool(name="p1pu", bufs=1, space="PSUM") as psu,
                    tc.tile_pool(name="p1pzr", bufs=1, space="PSUM") as pszr,
                ):
                    etil = ep.tile([P, NT, HL], BF)   # etilde per token
                    # stream all of x up front (resident bf16; host pre-cast).
                    # SWDGE lanes: keeps the HWDGE lanes free for xT loads.
                    for ti in range(NT):
                        nc.gpsimd.dma_start(xbf[:, ti, :],
                                            x_d[ti * P:(ti + 1) * P, :])
                    # stage-C weights: only needed after pass 1
                    nc.gpsimd.dma_start(selmat[:], selmat_d[:])
                    nc.gpsimd.dma_start(wvg_s[:], wvg_d[:])
                    nc.gpsimd.dma_start(w3_r[:], w3_d[:])
                    nc.gpsimd.dma_start(biasf[:], biasf_d[:])
                    for q in range(NQ):
                        for lt in range(TPQ):
                            ti = q * TPQ + lt
                            # stats on bf16 x
                            bns = sb.tile([P, 4, 6], F32, tag="bns", bufs=3)
                            for a in range(4):
                                nc.vector.bn_stats(bns[:, a, :],
                                                   xbf[:, ti, a * 512:(a + 1) * 512])
                            mv = sb.tile([P, 2], F32, tag="mv", bufs=3)
                            nc.vector.bn_aggr(mv[:], bns[:])
                            sig = sb.tile([P, 1], F32, tag="sig", bufs=3)
                            nc.scalar.activation(sig[:], mv[:, 1:2], AF.Sqrt,
                                                 bias=eps_col[:])
                            alpha = sb.tile([P, 1], F32, tag="alpha", bufs=3)
                            nc.vector.reciprocal(alpha[:], sig[:])
                            scl = sb.tile([P, 1], F32, tag="scl", bufs=3)
                            nc.vector.tensor_scalar_mul(scl[:], alpha[:], 1.0 / QSCALE)
                            nc.vector.tensor_copy(out=musig[:, ti, 0:1], in_=mv[:, 0:1])
                            nc.vector.tensor_copy(out=musig[:, ti, 1:2], in_=sig[:])
                            # transposed x tile [d, tok] (host-pretransposed fp8)
                            xbfT = sb.tile([P, KT, P], F8, tag="xbfT", bufs=3)
                            nc.sync.dma_start(xbfT[:], xT_d[:, ti, :, :])
                            # mu as a row (PE transpose)
                            mur_ps = ps.tile([1, P], F32, tag="sc", bufs=3,
                                             name=f"mur{ti}")
                            nc.tensor.matmul(mur_ps[:], musig[:, ti, 0:1],
                                             ident_bf[:], start=True, stop=True)
                            murow = sb.tile([1, P], BF, tag="murow", bufs=3)
                            nc.vector.tensor_copy(out=murow[:], in_=mur_ps[:])
                            # scoresT accumulation: rank-1 (-c*mu) then x.qhat
                            sc_ps = ps.tile([P, HL], F32, tag="sc", bufs=3,
                                            name=f"sc{ti}")
                            nc.tensor.matmul(sc_ps[:], murow[:], cneg[:],
                                             start=True, stop=False)
                            for j in range(KT // 2):
                                nc.tensor.matmul(
                                    sc_ps[:], xbfT[:, 2 * j:2 * j + 2, :],
                                    qhatT[:, 2 * j:2 * j + 2, :],
                                    start=False, stop=(j == KT // 2 - 1),
                                    perf_mode=mybir.MatmulPerfMode.DoubleRow)
                            # etilde = alpha * exp((alpha/QSCALE)*s)  (bf16)
                            eraw = sb.tile([P, HL], BF, tag="eraw", bufs=2)
                            nc.scalar.activation(eraw[:], sc_ps[:], AF.Exp,
                                                 scale=scl[:])
                            nc.vector.tensor_scalar_mul(etil[:, ti, :], eraw[:],
                                                        alpha[:])

                        # ---- u / zr sweep for this quarter ----
                        for mh in range(2):
                            psum_u = psu.tile([P, D], F32, tag="u", bufs=1,
                                              name=f"u{q}_{mh}")
                            zr_ps = pszr.tile([P, 2], F32, tag="zr", bufs=1,
                                              name=f"zr{q}_{mh}")
                            for lt in range(TPQ):
                                ti = q * TPQ + lt
                                lhs = etil[:, ti, mh * P:(mh + 1) * P]
                                for nch in range(NCH):
                                    nc.tensor.matmul(
                                        psum_u[:, nch * 512:(nch + 1) * 512],
                                        lhs, xbf[:, ti, nch * 512:(nch + 1) * 512],
                                        start=(lt == 0), stop=(lt == TPQ - 1),
                                        skip_group_check=True)
                                nc.tensor.matmul(zr_ps[:], lhs, musig[:, ti, :],
                                                 start=(lt == 0), stop=(lt == TPQ - 1),
                                                 skip_group_check=True)
                            if q == 0:
                                nc.vector.tensor_copy(out=u_acc[:, mh, :],
                                                      in_=psum_u[:])
                            else:
                                nc.vector.tensor_tensor(u_acc[:, mh, :],
                                                        u_acc[:, mh, :], psum_u[:],
                                                        mybir.AluOpType.add)
                            nc.vector.tensor_copy(out=zr_acc[:, mh, :, q],
                                                  in_=zr_ps[:])

                # ================= STAGE C =================
                with tc.tile_pool(name="c_sb", bufs=1) as csb:
                    # r, Z totals; M' = (u - r)/Z in place
                    zrt = csb.tile([P, 2, 2], F32)
                    nc.vector.tensor_reduce(zrt[:], zr_acc[:], mybir.AxisListType.X,
                                            mybir.AluOpType.add)
                    rzi = csb.tile([P, 2, 1], F32)
                    nc.vector.reciprocal(rzi[:], zrt[:, :, 1:2])
                    for mh in range(2):
                        nc.vector.tensor_scalar(u_acc[:, mh, :], u_acc[:, mh, :],
                                                zrt[:, mh, 0:1], rzi[:, mh, :],
                                                mybir.AluOpType.subtract,
                                                mybir.AluOpType.mult)
                    # mbar = per-head mean [H, D] (bf16)
                    mb_bf = csb.tile([H, D], BF)
                    with tc.tile_pool(name="c_ps_mb", bufs=1, space="PSUM") as cps0:
                        mb_ps = cps0.tile([H, D], F32)
                        for mh in range(2):
                            for nch in range(NCH):
                                nc.tensor.matmul(mb_ps[:, nch * 512:(nch + 1) * 512],
                                                 selmat[:, mh, :],
                                                 u_acc[:, mh, nch * 512:(nch + 1) * 512],
                                                 start=(mh == 0), stop=(mh == 1),
                                                 skip_group_check=True)
                        nc.scalar.copy(out=mb_bf[:], in_=mb_ps[:])
                    nc.sync.dma_start(mb_bounce[:], mb_bf[:])
                    nc.gpsimd.collective_compute(
                        "AllToAll", mybir.AluOpType.bypass, replica_groups=groups,
                        ins=[mb_bounce[:].opt()], outs=[m2_bounce[:].opt()])
                    # m2 rows = (batch b, local head h2); transpose -> [d, (b,h2)]
                    m2T = csb.tile([P, KT, H], BF)
                    nc.sync.dma_start_transpose(m2T[:], m2_bounce[:])
                    # cb[h2] = mb_h @ Wvg_slice block  -> transpose -> [j, b]
                    cbT = csb.tile([P, HPC, NB], BF)
                    with tc.tile_pool(name="c_ps_cb", bufs=1, space="PSUM") as cps1:
                        for h2 in range(HPC):
                            cb_ps = cps1.tile([NB, P], F32, tag="cbps", bufs=2)
                            for kt in range(KT):
                                lhs = m2T[:, kt, :].rearrange("p (b h) -> p h b", h=HPC)
                                nc.tensor.matmul(cb_ps[:], lhs[:, h2, :],
                                                 wvg_s[:, kt, h2 * P:(h2 + 1) * P],
                                                 start=(kt == 0), stop=(kt == KT - 1))
                            cb_sb = csb.tile([NB, P], BF, tag="cbsb", bufs=2)
                            nc.scalar.copy(out=cb_sb[:], in_=cb_ps[:])
                            ct_ps = cps1.tile([P, NB], F32, tag="ctps", bufs=2)
                            nc.tensor.matmul(ct_ps[:], cb_sb[:], ident_bf[:NB, :NB],
                                             start=True, stop=True)
                            nc.scalar.copy(out=cbT[:, h2, :], in_=ct_ps[:])
                    # partial out rows for all batches: po = cb @ W3_rows + biasf/NB
                    with tc.tile_pool(name="c_ps_po", bufs=1, space="PSUM") as cps2:
                        po_ps = cps2.tile([NB, D], F32)
                        for h2 in range(HPC):
                            for nch in range(NCH):
                                nc.tensor.matmul(po_ps[:, nch * 512:(nch + 1) * 512],
                                                 cbT[:, h2, :],
                                                 w3_r[:, h2, nch * 512:(nch + 1) * 512],
                                                 start=(h2 == 0), stop=False,
                                                 skip_group_check=True)
                        for nch in range(NCH):
                            nc.tensor.matmul(po_ps[:, nch * 512:(nch + 1) * 512],
                                             ones8_bf[:],
                                             biasf[:, nch * 512:(nch + 1) * 512],
                                             start=False, stop=(nch == NCH - 1),
                                             skip_group_check=True)
                        for half in range(2):
                            po_sb = csb.tile([NB, D // 2], F32, tag="posb", bufs=1)
                            nc.scalar.copy(out=po_sb[:],
                                           in_=po_ps[:, half * 1024:(half + 1) * 1024])
                            nc.sync.dma_start(
                                po_bounce[:, half * 1024:(half + 1) * 1024], po_sb[:])
                    nc.gpsimd.collective_compute(
                        "ReduceScatter", mybir.AluOpType.add, replica_groups=groups,
                        ins=[po_bounce[:].opt()], outs=[ob_bounce[:].opt()])
                    # broadcast own out row to 128 partitions (bf16)
                    ob_bf = csb.tile([1, D], BF)
                    nc.gpsimd.dma_start(ob_bf[:], ob_bounce[:])  # f32->bf16 cast
                    obb = res.tile([P, D], BF)
                    with tc.tile_pool(name="c_ps_bc", bufs=1, space="PSUM") as cps3:
                        bc_ps = cps3.tile([P, D], F32)
                        for nch in range(NCH):
                            nc.tensor.matmul(bc_ps[:, nch * 512:(nch + 1) * 512],
                                             onesrow_bf[:],
                                             ob_bf[:, nch * 512:(nch + 1) * 512],
                                             start=True, stop=True,
                                             skip_group_check=True)
                        nc.scalar.copy(out=obb[:], in_=bc_ps[:])

                # ================= PASS 2 (residual, no x re-read) =========
                with tc.tile_pool(name="res2", bufs=1) as r2:
                    for ti in range(NT):
                        yt = r2.tile([P, D], F32, tag="yt", bufs=3)
                        # vector is ~1.4x faster than gpsimd: give it 2 of 3
                        eng = nc.gpsimd if ti % 3 == 2 else nc.vector
                        eng.tensor_tensor(yt[:], xbf[:, ti, :], obb[:],
                                          mybir.AluOpType.add)
                        nc.sync.dma_start(y_d[ti * P:(ti + 1) * P, :], yt[:])

    nc.compile()
    return nc


@functools.lru_cache(maxsize=2)
def _built(n_cores: int, S: int):
    return _build(n_cores, S)


def _host_prep(inputs, n_cores: int):
    """Weight folding on host. Returns (global_map, per_core_maps)."""
    NB = n_cores
    HPC = H // NB
    SL = D // NB
    f32 = np.float32
    bf16 = ml_dtypes.bfloat16
    f8 = ml_dtypes.float8_e4m3

    x_all = np.ascontiguousarray(np.asarray(inputs["hidden_states"], dtype=f32))
    g = np.asarray(inputs["ln_g"], dtype=f32)
    b_ln = np.asarray(inputs["ln_b"], dtype=f32)
    lat = np.asarray(inputs["latents"], dtype=f32)
    w_lq = np.asarray(inputs["w_lq"], dtype=f32)
    b_lq = np.asarray(inputs["b_lq"], dtype=f32)
    w_k = np.asarray(inputs["w_k"], dtype=f32)
    w_v = np.asarray(inputs["w_v"], dtype=f32)
    b_v = np.asarray(inputs["b_v"], dtype=f32)
    w_lv = np.asarray(inputs["w_lv"], dtype=f32)
    b_lv = np.asarray(inputs["b_lv"], dtype=f32)
    w_out = np.asarray(inputs["w_out"], dtype=f32)
    b_out = np.asarray(inputs["b_out"], dtype=f32)

    q_full = lat @ w_lq + b_lq                      # [L, D]
    qhatT = np.empty((D, HL), f32)
    for h in range(H):
        qh = q_full[:, HD * h:HD * (h + 1)]          # [L, 128]
        qhatT[:, L * h:L * (h + 1)] = w_k[:, HD * h:HD * (h + 1)] @ qh.T
    qhatT *= g[:, None] * INV_SQRT_HD               # fold 1/sqrt(hd)
    c_vec = qhatT.sum(axis=0)                        # [HL] (already scaled)

    def tile_rows(a):  # [D, N] -> [P, KT, N] with d = t*128 + p
        return np.ascontiguousarray(a.reshape(-1, P, a.shape[-1]).transpose(1, 0, 2))

    qhatT_t = tile_rows(qhatT * 128.0).astype(f8)      # QSCALE
    cneg = (-c_vec * 128.0)[None, :].astype(bf16)

    selmat = np.zeros((P, 2, H), f32)
    for mh in range(2):
        for p in range(P):
            selmat[p, mh, (mh * P + p) // L] = 1.0 / L

    wvg = w_v * g[:, None]
    w3 = w_lv @ w_out                                # folded Wlv@Wout [D, D]
    bv_fold = b_v + b_ln @ w_v
    biasf_full = ((bv_fold @ w_lv + b_lv) @ w_out + b_out) / NB

    global_map = {
        "qhatT": qhatT_t, "cneg": cneg, "selmat": selmat,
        "biasf": np.ascontiguousarray(biasf_full[None, :].astype(bf16)),
    }
    per_core = []
    for c in range(NB):
        sl = slice(SL * c, SL * (c + 1))
        wvg_s = tile_rows(wvg[:, sl]).astype(bf16)               # [P, KT, SL]
        w3_rows = np.ascontiguousarray(
            w3[sl, :].reshape(HPC, P, D).transpose(1, 0, 2)).astype(bf16)
        xc = x_all[c].astype(bf16)
        S = xc.shape[0]
        # xT[p, ti, kt, j] = x[ti*128+j, kt*128+p]  (fp8 for DoubleRow scores)
        xT = np.ascontiguousarray(
            x_all[c].astype(f8).reshape(S // P, P, D // P, P).transpose(3, 0, 2, 1))
        per_core.append({
            "x": np.ascontiguousarray(xc),
            "xT": xT, "wvg_s": wvg_s, "w3_r": w3_rows,
        })
    return global_map, per_core


def kernel(**inputs) -> np.ndarray:
    NB = 8
    x_all = np.asarray(inputs["hidden_states"])
    B, S, D_ = x_all.shape
    assert D_ == D and B == NB
    nc = _built(NB, S)
    global_map, per_core = _host_prep(inputs, NB)
    in_maps = [{**global_map, **pc} for pc in per_core]
    res = run_bass_kernel_spmd(nc, in_maps, list(range(NB)))
    out = np.stack([res.results[i]["y"] for i in range(NB)], axis=0)
    return out.astype(np.float32)
